# revision 3
# baseline (speedup 1.0000x reference)
"""Trainium2 Bass kernel for RecursiveMamba130M.

Math: the complex SSM state never needs materializing. With
  R = cos(theta) + j sin(theta),  Bc = Br + j Bi,  Cc = Cr + j Ci,
the per-loop output collapses to
  y_i[t, f] = sum_{k<=i} G_{i-k}[f] * u_k[t, f],   u_k = h_k @ W_in^T
where G_m[f] = sum_s Re(Cc * R^m * Bc).

Sharding: fully data-parallel over the 1024 sequence positions
(128 tokens per core, no collectives); small weights replicated.

Per-core program, all matmul/elementwise data in bf16 (fp32 PSUM
accumulation and fp32 norm statistics):
  loop i in 0..3:
    hT   = PE-transpose(h) -> bf16 PSUM (bank-padded) -> DVE evac
    u    = h @ W_in^T      (PE, 18 N=512 matmuls, chunk-major)
    u_sb = ACT evac per chunk; y = G0*u (+acc_i) on DVE per chunk
    acc_j += G_{j-i}*u     (DVE mul / GpSimd for slack lags)
    yT   = PE-transpose(y) -> bf16 PSUM -> DVE evac per chunk
    z    = y @ out_proj^T  (PE, two accumulation groups: 512/256)
    norm tail with per-chunk partial sums and the identity
      sum w^2 = rs_z^2*sum z^2 + 2*rs_z*sum(z*h) + sum h^2
"""

import numpy as np
import ml_dtypes

import concourse.bass as bass
import concourse.tile as tile
from concourse.bacc import Bacc
from concourse import masks, mybir
from concourse.bass_utils import run_bass_kernel_spmd

T = 128          # tokens per core
D = 768          # d_model
F = 1536         # 2 * d_model
NL = 4           # reasoning loops
NCORES = 8
EPS = 1e-6

f32 = mybir.dt.float32
bf16 = mybir.dt.bfloat16
AL = mybir.AluOpType
AF = mybir.ActivationFunctionType

_CACHE = {}


def build_nc():
    nc = Bacc()
    x_d = nc.dram_tensor("x_in", [T, D], bf16, kind="ExternalInput")
    winT_d = nc.dram_tensor("winT", [D, F], bf16, kind="ExternalInput")
    woutT_d = nc.dram_tensor("woutT", [F, D], bf16, kind="ExternalInput")
    g4_d = nc.dram_tensor("g4", [NL, F], bf16, kind="ExternalInput")
    s4_d = nc.dram_tensor("s4", [NL, D], bf16, kind="ExternalInput")
    out_d = nc.dram_tensor("x_out", [T, D], f32, kind="ExternalOutput")

    with tile.TileContext(nc) as tc:
        with (
            tc.tile_pool(name="wpool", bufs=1) as wpool,
            tc.tile_pool(name="apool", bufs=1) as apool,
            tc.tile_pool(name="work", bufs=2) as work,
            tc.tile_pool(name="scal", bufs=1) as scal,
            tc.tile_pool(name="ps_t", bufs=1, space="PSUM") as ps_t,
            tc.tile_pool(name="ps_u", bufs=1, space="PSUM") as ps_u,
            tc.tile_pool(name="ps_z", bufs=1, space="PSUM") as ps_z,
        ):
            # ---------- constants / weights ----------
            ident = wpool.tile([128, 128], bf16, tag="ident")
            masks.make_identity(nc, ident[:])
            ones1 = wpool.tile([1, 128], bf16, tag="ones1")
            nc.vector.memset(ones1[:].bitcast(mybir.dt.uint32), 0x3F803F80)
            eps_t = wpool.tile([T, 1], f32, tag="eps_t")
            nc.vector.memset(eps_t[:], EPS)

            x_sb = wpool.tile([T, D], bf16, tag="x_sb")
            nc.sync.dma_start(x_sb[:], x_d[:, :])

            # W_in^T as one [128, 6, F] tile, single DMA
            winT_sb = wpool.tile([128, 6, F], bf16, tag="winT_sb")
            nc.sync.dma_start(
                winT_sb[:], winT_d.rearrange("(k p) n -> p k n", p=128))

            # W_out^T as one [128, 12, D] tile, single DMA
            woutT_sb = wpool.tile([128, 12, D], bf16, tag="woutT_sb")
            nc.sync.dma_start(
                woutT_sb[:], woutT_d.rearrange("(k p) n -> p k n", p=128))

            # step_emb broadcast tiles [T, D] via K=1 matmul
            Sb = []
            for i in range(NL):
                sr = work.tile([1, D], bf16, tag="s_row", bufs=2,
                               name=f"s_row{i}")
                nc.sync.dma_start(sr[:], s4_d[i:i + 1, :])
                sb_ps = ps_z.tile([T, D], f32, tag="z")
                for off, nn in ((0, 512), (512, 256)):
                    nc.tensor.matmul(
                        sb_ps[:, off:off + nn],
                        ones1[:, :],
                        sr[:, off:off + nn],
                        start=True, stop=True,
                    )
                sb = wpool.tile([T, D], bf16, tag=f"Sb{i}")
                nc.scalar.copy(sb[:], sb_ps[:])
                Sb.append(sb)

            # G broadcast tiles [T, F]
            Gb = []
            for m in range(NL):
                gr = work.tile([1, F], bf16, tag="g_row", bufs=2,
                               name=f"g_row{m}")
                nc.sync.dma_start(gr[:], g4_d[m:m + 1, :])
                gb_ps = ps_u.tile([T, F], f32, tag="u")
                for n in range(3):
                    nc.tensor.matmul(
                        gb_ps[:, 512 * n:512 * (n + 1)],
                        ones1[:, :],
                        gr[:, 512 * n:512 * (n + 1)],
                        start=True, stop=True,
                    )
                gb = wpool.tile([T, F], bf16, tag=f"Gb{m}")
                nc.scalar.copy(gb[:], gb_ps[:])
                Gb.append(gb)

            # ---------- h0 = x + Sb0 ----------
            h = work.tile([T, D], bf16, tag="h", bufs=2)
            nc.vector.tensor_add(h[:], x_sb[:], Sb[0][:])
            ssh = scal.tile([T, 1], f32, tag="ssh", bufs=2)
            sq_scr0 = work.tile([T, D], bf16, tag="scr", bufs=2)
            nc.scalar.activation(sq_scr0[:], h[:], AF.Square, accum_out=ssh[:])

            accs = {}
            for j in (1, 2, 3):
                accs[j] = apool.tile([T, F], bf16, tag=f"acc{j}",
                                     name=f"acc{j}")

            # ---------- main loop ----------
            for i in range(NL):
                # hT (stationary for MM1), bf16 PSUM padded to one bank per
                # 384-col half so DVE evacuation overlaps later transposes
                hT_ps = ps_t.tile([T, 2, 1024], bf16, tag="t")
                for k in range(6):
                    nc.tensor.transpose(
                        hT_ps[:, k // 3, 128 * (k % 3):128 * (k % 3 + 1)],
                        h[:, 128 * k:128 * (k + 1)],
                        ident[:],
                    )
                hT_sb = work.tile([T, D], bf16, tag="hT_sb", bufs=1)
                for half in range(2):
                    nc.vector.tensor_copy(
                        hT_sb[:, 384 * half:384 * (half + 1)],
                        hT_ps[:, half, 0:384])

                # MM1: u = h @ W_in^T   [T, F], chunk-major so each 512-wide
                # chunk retires early and the copy/combine overlaps the PE
                u_ps = ps_u.tile([T, F], f32, tag="u")
                for n in range(3):
                    for k in range(6):
                        nc.tensor.matmul(
                            u_ps[:, 512 * n:512 * (n + 1)],
                            hT_sb[:, 128 * k:128 * (k + 1)],
                            winT_sb[:, k, 512 * n:512 * (n + 1)],
                            start=(k == 0), stop=(k == 5),
                        )

                u_sb = work.tile([T, F], bf16, tag="u_sb", bufs=1)
                y = work.tile([T, F], bf16, tag="y", bufs=1)
                for n in range(3):
                    sl = slice(512 * n, 512 * (n + 1))
                    nc.scalar.copy(u_sb[:, sl], u_ps[:, sl])
                    nc.vector.tensor_mul(y[:, sl], u_sb[:, sl], Gb[0][:, sl])
                    if i > 0:
                        nc.vector.tensor_add(y[:, sl], y[:, sl],
                                             accs[i][:, sl])

                # acc updates (off critical path): acc_j += G_{j-i} * u
                # GpSimd (slow, SBUF-only) takes only deep-slack terms
                for j in range(i + 1, NL):
                    m = j - i
                    if i == 0:
                        eng = nc.vector if j == 1 else nc.gpsimd
                        eng.tensor_mul(accs[j][:], u_sb[:], Gb[m][:])
                    elif j == i + 1:
                        tmp_a = work.tile([T, F], bf16, tag="tmp_a", bufs=2)
                        nc.vector.tensor_mul(tmp_a[:], u_sb[:], Gb[m][:])
                        nc.vector.tensor_add(accs[j][:], accs[j][:],
                                             tmp_a[:])
                    else:
                        tmp_b = work.tile([T, F], bf16, tag="tmp_b", bufs=2)
                        nc.gpsimd.tensor_mul(tmp_b[:], u_sb[:], Gb[m][:])
                        nc.gpsimd.tensor_add(accs[j][:], accs[j][:],
                                             tmp_b[:])

                # yT (stationary for MM2)
                yT_ps = ps_t.tile([T, F], bf16, tag="t")
                for c in range(12):
                    nc.tensor.transpose(
                        yT_ps[:, 128 * c:128 * (c + 1)],
                        y[:, 128 * c:128 * (c + 1)],
                        ident[:],
                    )
                yT_sb = work.tile([T, F], bf16, tag="yT_sb", bufs=1)
                for n in range(3):
                    sl = slice(512 * n, 512 * (n + 1))
                    nc.vector.tensor_copy(yT_sb[:, sl], yT_ps[:, sl])

                # MM2: z = y @ out_proj^T   [T, D]; one accumulation group
                # per z chunk so the 512-wide chunk's norm ops start early
                z_ps = ps_z.tile([T, D], f32, tag="z")
                for off, nn in ((0, 512), (512, 256)):
                    for c in range(12):
                        nc.tensor.matmul(
                            z_ps[:, off:off + nn],
                            yT_sb[:, 128 * c:128 * (c + 1)],
                            woutT_sb[:, c, off:off + nn],
                            start=(c == 0), stop=(c == 11),
                        )

                # norm tail: per-chunk partial sums overlap the 256-chunk MMs
                ssz_p = []
                szh_p = []
                for ci, (off, nn) in enumerate(((0, 512), (512, 256))):
                    sl = slice(off, off + nn)
                    ssp = scal.tile([T, 1], f32, tag=f"ssz{ci}")
                    scr = work.tile([T, 512], bf16, tag="scr5", bufs=2)
                    nc.scalar.activation(scr[:, 0:nn], z_ps[:, sl], AF.Square,
                                         accum_out=ssp[:])
                    ssz_p.append(ssp)
                    shp = scal.tile([T, 1], f32, tag=f"szh{ci}")
                    zscr = work.tile([T, 512], bf16, tag="zscr", bufs=2)
                    nc.vector.scalar_tensor_tensor(
                        out=zscr[:, 0:nn], in0=z_ps[:, sl], scalar=2.0,
                        in1=h[:, sl], op0=AL.mult, op1=AL.mult,
                        accum_out=shp[:],
                    )
                    szh_p.append(shp)
                ss_z = scal.tile([T, 1], f32, tag="ss_z")
                nc.vector.tensor_add(ss_z[:], ssz_p[0][:], ssz_p[1][:])
                szh2 = scal.tile([T, 1], f32, tag="szh2")
                nc.vector.tensor_add(szh2[:], szh_p[0][:], szh_p[1][:])

                sq_z = scal.tile([T, 1], f32, tag="sq_z")
                nc.scalar.activation(sq_z[:], ss_z[:], AF.Sqrt,
                                     bias=eps_t[:, :], scale=1.0 / D)
                rs_z = scal.tile([T, 1], f32, tag="rs_z")
                nc.vector.reciprocal(rs_z[:], sq_z[:])

                # ss_w = rs_z^2*ss_z + 2*rs_z*szh + ssh  (tiny [T,1] ops)
                v1 = scal.tile([T, 1], f32, tag="v1")
                nc.vector.scalar_tensor_tensor(
                    out=v1[:], in0=ss_z[:], scalar=rs_z[:, :], in1=szh2[:],
                    op0=AL.mult, op1=AL.add,
                )
                ss_w = scal.tile([T, 1], f32, tag="ss_w")
                nc.vector.scalar_tensor_tensor(
                    out=ss_w[:], in0=v1[:], scalar=rs_z[:, :], in1=ssh[:],
                    op0=AL.mult, op1=AL.add,
                )
                sq_w = scal.tile([T, 1], f32, tag="sq_w")
                nc.scalar.activation(sq_w[:], ss_w[:], AF.Sqrt,
                                     bias=eps_t[:, :], scale=1.0 / D)
                rs_w = scal.tile([T, 1], f32, tag="rs_w")
                nc.vector.reciprocal(rs_w[:], sq_w[:])

                # w = z * rs_z + h
                w = work.tile([T, D], bf16, tag="w", bufs=1)
                nc.vector.scalar_tensor_tensor(
                    out=w[:], in0=z_ps[:], scalar=rs_z[:, :], in1=h[:],
                    op0=AL.mult, op1=AL.add,
                )

                if i < NL - 1:
                    h_next = work.tile([T, D], bf16, tag="h", bufs=2)
                    nc.vector.scalar_tensor_tensor(
                        out=h_next[:], in0=w[:], scalar=rs_w[:, :],
                        in1=Sb[i + 1][:], op0=AL.mult, op1=AL.add,
                    )
                    h = h_next
                    ssh = scal.tile([T, 1], f32, tag="ssh", bufs=2)
                    sq_scrh = work.tile([T, D], bf16, tag="scr", bufs=2)
                    nc.scalar.activation(sq_scrh[:], h[:], AF.Square,
                                         accum_out=ssh[:])
                else:
                    out_f = work.tile([T, D], f32, tag="out_f", bufs=1)
                    nc.vector.tensor_scalar_mul(out_f[:], w[:], rs_w[:, :])
                    nc.sync.dma_start(out_d[:, :], out_f[:])

    nc.compile()
    return nc


def _host_prep(x, in_proj_base, lora_A, lora_B, A_theta, B_real, B_imag,
               C_real, C_imag, out_proj_w, step_emb):
    W_in = in_proj_base.astype(np.float64) + 2.0 * (
        lora_B.astype(np.float64) @ lora_A.astype(np.float64))
    winT = np.ascontiguousarray(W_in.T).astype(ml_dtypes.bfloat16)
    woutT = np.ascontiguousarray(out_proj_w.T).astype(ml_dtypes.bfloat16)

    th = A_theta.astype(np.float64)
    P = (C_real.astype(np.float64) * B_real.astype(np.float64)
         - C_imag.astype(np.float64) * B_imag.astype(np.float64))
    Q = (C_real.astype(np.float64) * B_imag.astype(np.float64)
         + C_imag.astype(np.float64) * B_real.astype(np.float64))
    g4 = np.stack([
        (P * np.cos(m * th) - Q * np.sin(m * th)).sum(-1).reshape(-1)
        for m in range(NL)
    ]).astype(ml_dtypes.bfloat16)                           # [4, 1536]
    s4 = np.ascontiguousarray(step_emb).astype(ml_dtypes.bfloat16)
    return winT, woutT, g4, s4


def kernel(x, in_proj_base, lora_A, lora_B, A_theta, B_real, B_imag,
           C_real, C_imag, out_proj_w, mixer_norm_w, loop_norm_w, step_emb,
           _trace=False):
    x = np.asarray(x, dtype=np.float32)
    winT, woutT, g4, s4 = _host_prep(
        np.asarray(x), np.asarray(in_proj_base), np.asarray(lora_A),
        np.asarray(lora_B), np.asarray(A_theta), np.asarray(B_real),
        np.asarray(B_imag), np.asarray(C_real), np.asarray(C_imag),
        np.asarray(out_proj_w), np.asarray(step_emb))
    # mixer_norm_w / loop_norm_w are ones per the problem spec; rmsnorm weight
    # multiplies are identity and omitted on device.

    if "nc" not in _CACHE:
        _CACHE["nc"] = build_nc()
    nc = _CACHE["nc"]

    x16 = x.astype(ml_dtypes.bfloat16)
    shared = {"winT": winT, "woutT": woutT, "g4": g4, "s4": s4}
    in_maps = [
        {**shared, "x_in": np.ascontiguousarray(x16[0, T * c:T * (c + 1), :])}
        for c in range(NCORES)
    ]
    res = run_bass_kernel_spmd(nc, in_maps, list(range(NCORES)), trace=_trace)
    out = np.concatenate(
        [np.asarray(res.results[c]["x_out"]) for c in range(NCORES)], axis=0)
    if _trace:
        _CACHE["last_result"] = res
    return out[None, :, :].astype(np.float32)


# revision 8
# speedup vs baseline: 1.4356x; 1.4356x over previous
"""Trainium2 Bass kernel for RecursiveMamba130M.

Math: the complex SSM state never needs materializing. With
  R = cos(theta) + j sin(theta),  Bc = Br + j Bi,  Cc = Cr + j Ci,
the per-loop output collapses to
  y_i[t, f] = sum_{k<=i} G_{i-k}[f] * u_k[t, f],   u_k = h_k @ W_in^T
where G_m[f] = sum_s Re(Cc * R^m * Bc).

Two more algebraic folds keep the PE dense:
  * h_{i+1} = rs_w*w + step  =>  u_{i+1} = rs_w*(w @ W_in^T) + step@W_in^T.
    The per-token scale rs_w commutes through the matmul, so the PE
    transposes w (available right after rs_z) instead of h, and the
    rs_w scale rides the ACT PSUM->SBUF evacuation for free.
  * The step@W_in^T terms are constant rows; their contribution to z is
    zdb_i = (sum_k G_{i-k}*sW_k) @ W_out^T, injected into MM2's PSUM
    accumulation as a rank-1 ones-matmul. y on device is pure
    G0*u' + acc.
  * sum w^2 = rs_z^2*sum z^2 + 2*rs_z*sum(z*h) + sum h^2 (norm tail
    shortened; sum(z*h) on DVE in parallel with sum z^2 on ACT).

Sharding: data-parallel over the 1024 positions (128 tokens/core, no
collectives); weights replicated, all matmul data bf16 (fp32 PSUM).
All y transposes ride the idle DMA engines (xbar dma_start_transpose);
w transposes use the PE during the norm tail.
"""

import numpy as np
import ml_dtypes

import concourse.bass as bass
import concourse.tile as tile
from concourse.bacc import Bacc
from concourse import masks, mybir
from concourse.bass_utils import run_bass_kernel_spmd

T = 128          # tokens per core
D = 768          # d_model
F = 1536         # 2 * d_model
NL = 4           # reasoning loops
NCORES = 8
EPS = 1e-6

f32 = mybir.dt.float32
bf16 = mybir.dt.bfloat16
AL = mybir.AluOpType
AF = mybir.ActivationFunctionType

_CACHE = {}


def _act_rsqrt(nc, out, in_, bias_ap, scale):
    """out = Rsqrt(in_*scale + bias) on ScalarE.

    The bass wrapper refuses Rsqrt over precision concerns far below this
    kernel's 2e-2 tolerance; emit the instruction directly.
    """
    eng = nc.scalar
    ins = [
        eng.lower_ap(in_),
        eng.lower_ap(bias_ap),
        mybir.ImmediateValue(dtype=mybir.dt.float32, value=float(scale)),
        mybir.ImmediateValue(dtype=mybir.dt.float32, value=0.0),
    ]
    return eng.add_instruction(
        mybir.InstActivation(
            name=nc.get_next_instruction_name(),
            func=AF.Rsqrt,
            ins=ins,
            outs=[eng.lower_ap(out)],
        )
    )


def build_nc():
    nc = Bacc()
    x_d = nc.dram_tensor("x_in", [T, D], bf16, kind="ExternalInput")
    winT_d = nc.dram_tensor("winT", [D, F], bf16, kind="ExternalInput")
    woutT_d = nc.dram_tensor("woutT", [F, D], bf16, kind="ExternalInput")
    g4_d = nc.dram_tensor("g4", [NL, F], bf16, kind="ExternalInput")
    s4_d = nc.dram_tensor("s4", [NL, D], bf16, kind="ExternalInput")
    zdb_d = nc.dram_tensor("zdb", [NL, D], bf16, kind="ExternalInput")
    out_d = nc.dram_tensor("x_out", [T, D], f32, kind="ExternalOutput")

    with tile.TileContext(nc) as tc:
        with (
            tc.tile_pool(name="wpool", bufs=1) as wpool,
            tc.tile_pool(name="apool", bufs=1) as apool,
            tc.tile_pool(name="work", bufs=2) as work,
            tc.tile_pool(name="scal", bufs=1) as scal,
            tc.tile_pool(name="ps_u", bufs=1, space="PSUM") as ps_u,
            tc.tile_pool(name="ps_z", bufs=1, space="PSUM") as ps_z,
            tc.tile_pool(name="ps_t", bufs=1, space="PSUM") as ps_t,
        ):
            # ---------- constants ----------
            ident = wpool.tile([128, 128], bf16, tag="ident")
            masks.make_identity(nc, ident[:])
            ones1 = wpool.tile([1, 128], bf16, tag="ones1")
            nc.vector.memset(ones1[:].bitcast(mybir.dt.uint32), 0x3F803F80)
            eps_t = wpool.tile([T, 1], f32, tag="eps_t")
            nc.vector.memset(eps_t[:], EPS)

            # ---------- DMAs (order = priority: x + rows, then weights) ----
            x_sb = wpool.tile([T, D], bf16, tag="x_sb")
            nc.sync.dma_start(x_sb[:], x_d[:, :])

            s_rows, g_rows, zdb_rows = [], [], []
            for i in range(NL):
                sr = wpool.tile([1, D], bf16, tag=f"s_row{i}",
                                name=f"s_row{i}")
                nc.sync.dma_start(sr[:], s4_d[i:i + 1, :])
                s_rows.append(sr)
                gr = wpool.tile([1, F], bf16, tag=f"g_row{i}",
                                name=f"g_row{i}")
                nc.sync.dma_start(gr[:], g4_d[i:i + 1, :])
                g_rows.append(gr)
                zr = wpool.tile([1, D], bf16, tag=f"zdb_row{i}",
                                name=f"zdb_row{i}")
                nc.sync.dma_start(zr[:], zdb_d[i:i + 1, :])
                zdb_rows.append(zr)

            winT_sb = wpool.tile([128, 6, F], bf16, tag="winT_sb")
            for k in range(6):
                nc.sync.dma_start(winT_sb[:, k, :],
                                  winT_d[128 * k:128 * (k + 1), :])
            woutT_sb = wpool.tile([128, 12, D], bf16, tag="woutT_sb")
            for g in range(4):
                nc.sync.dma_start(
                    woutT_sb[:, 3 * g:3 * (g + 1), :],
                    woutT_d[384 * g:384 * (g + 1), :].rearrange(
                        "(k p) n -> p k n", p=128))

            # ---------- broadcast tiles via K=1 ones-matmul ----------
            # Sb[i] [T, D] built in the z PSUM tiles; Gb[m] [T, F] in u tiles
            zA_chunks = ((0, 512), (512, 256))
            Sb = []
            for i in range(NL):
                sb = wpool.tile([T, D], bf16, tag=f"Sb{i}", name=f"Sb{i}")
                for ci, (off, nn) in enumerate(zA_chunks):
                    zt = ps_z.tile([T, nn], f32, tag=f"z{ci}",
                                   name=f"sbz{i}{ci}")
                    nc.tensor.matmul(zt[:], ones1[:, :],
                                     s_rows[i][:, off:off + nn],
                                     start=True, stop=True)
                    nc.scalar.copy(sb[:, off:off + nn], zt[:])
                Sb.append(sb)

            Gb = []
            for m in range(NL):
                gb = wpool.tile([T, F], bf16, tag=f"Gb{m}", name=f"Gb{m}")
                for n in range(3):
                    ut = ps_u.tile([T, 512], f32, tag=f"u{n}",
                                   name=f"gbu{m}{n}")
                    nc.tensor.matmul(ut[:], ones1[:, :],
                                     g_rows[m][:, 512 * n:512 * (n + 1)],
                                     start=True, stop=True)
                    nc.scalar.copy(gb[:, 512 * n:512 * (n + 1)], ut[:])
                Gb.append(gb)

            # ---------- h0 = x + Sb0; transpose h0 on PE ----------
            h = work.tile([T, D], bf16, tag="h", bufs=2)
            nc.vector.tensor_add(h[:], x_sb[:], Sb[0][:])
            ssh = scal.tile([T, 1], f32, tag="ssh", bufs=2)
            scr0 = work.tile([T, D], bf16, tag="scr", bufs=2)
            nc.scalar.activation(scr0[:], h[:], AF.Square, accum_out=ssh[:])

            def transpose_to_sbuf(src, label):
                """PE-transpose src [T, 768] -> [T, 768] bf16 (k-tile major),
                bank-padded PSUM halves so DVE evacuation overlaps."""
                t_ps = ps_t.tile([T, 2, 1024], bf16, tag="t",
                                 name=f"tps_{label}")
                for k in range(6):
                    nc.tensor.transpose(
                        t_ps[:, k // 3, 128 * (k % 3):128 * (k % 3 + 1)],
                        src[:, 128 * k:128 * (k + 1)],
                        ident[:],
                    )
                t_sb = work.tile([T, D], bf16, tag="pT_sb", bufs=2,
                                 name=f"tsb_{label}")
                for half in range(2):
                    nc.vector.tensor_copy(
                        t_sb[:, 384 * half:384 * (half + 1)],
                        t_ps[:, half, 0:384])
                return t_sb

            pT_sb = transpose_to_sbuf(h, "h0")

            accs = {}
            for j in (1, 2, 3):
                accs[j] = apool.tile([T, F], bf16, tag=f"acc{j}",
                                     name=f"acc{j}")

            rs_w = None
            # ---------- main loop ----------
            for i in range(NL):
                # MM1: p = (h|w) @ W_in^T, chunk-major for early evacuation
                u_ps = []
                for n in range(3):
                    ut = ps_u.tile([T, 512], f32, tag=f"u{n}",
                                   name=f"u{i}_{n}")
                    for k in range(6):
                        nc.tensor.matmul(
                            ut[:],
                            pT_sb[:, 128 * k:128 * (k + 1)],
                            winT_sb[:, k, 512 * n:512 * (n + 1)],
                            start=(k == 0), stop=(k == 5),
                        )
                    u_ps.append(ut)

                # u' = rs_w * p rides the ACT evacuation (plain copy at i=0)
                u_sb = work.tile([T, F], bf16, tag="u_sb", bufs=2)
                y = work.tile([T, F], bf16, tag="y", bufs=2)
                yT_sb = work.tile([128, 12, 128], bf16, tag="yT_sb", bufs=2)
                for n in range(3):
                    sl = slice(512 * n, 512 * (n + 1))
                    if i == 0:
                        nc.scalar.copy(u_sb[:, sl], u_ps[n][:])
                    else:
                        nc.scalar.activation(u_sb[:, sl], u_ps[n][:],
                                             AF.Copy, scale=rs_w[:, :])
                    nc.vector.tensor_mul(y[:, sl], u_sb[:, sl], Gb[0][:, sl])
                    if i > 0:
                        nc.vector.tensor_add(y[:, sl], y[:, sl],
                                             accs[i][:, sl])
                    # yT for this chunk rides the idle DMA xbar
                    nc.sync.dma_start_transpose(
                        yT_sb[:, 4 * n:4 * (n + 1), :], y[:, sl])

                # acc updates (DVE, off critical path)
                for j in range(i + 1, NL):
                    m = j - i
                    if i == 0:
                        nc.vector.tensor_mul(accs[j][:], u_sb[:], Gb[m][:])
                    else:
                        tmp_a = work.tile([T, F], bf16, tag="tmp_a", bufs=2)
                        nc.vector.tensor_mul(tmp_a[:], u_sb[:], Gb[m][:])
                        nc.vector.tensor_add(accs[j][:], accs[j][:],
                                             tmp_a[:])

                # MM2: z = y @ out_proj^T + ones x zdb_i, A(512) then B(256)
                z_ps = []
                for ci, (off, nn) in enumerate(zA_chunks):
                    zt = ps_z.tile([T, nn], f32, tag=f"z{ci}",
                                   name=f"z{i}_{ci}")
                    if i > 0:
                        nc.tensor.matmul(zt[:], ones1[:, :],
                                         zdb_rows[i][:, off:off + nn],
                                         start=True, stop=False)
                    for c in range(12):
                        nc.tensor.matmul(
                            zt[:],
                            yT_sb[:, c, :],
                            woutT_sb[:, c, off:off + nn],
                            start=(c == 0 and i == 0), stop=(c == 11),
                        )
                    z_ps.append(zt)

                # norm tail: per-chunk partials (A's run under B's matmuls)
                ssz_p, szh_p = [], []
                for ci, (off, nn) in enumerate(zA_chunks):
                    ssp = scal.tile([T, 1], f32, tag=f"ssz{ci}")
                    scr = work.tile([T, 512], bf16, tag="scr5", bufs=2)
                    nc.scalar.activation(scr[:, 0:nn], z_ps[ci][:], AF.Square,
                                         accum_out=ssp[:])
                    ssz_p.append(ssp)
                    shp = scal.tile([T, 1], f32, tag=f"szh{ci}")
                    zscr = work.tile([T, 512], bf16, tag="zscr", bufs=2)
                    nc.vector.scalar_tensor_tensor(
                        out=zscr[:, 0:nn], in0=z_ps[ci][:], scalar=2.0,
                        in1=h[:, off:off + nn], op0=AL.mult, op1=AL.mult,
                        accum_out=shp[:],
                    )
                    szh_p.append(shp)
                ss_z = scal.tile([T, 1], f32, tag="ss_z")
                nc.vector.tensor_add(ss_z[:], ssz_p[0][:], ssz_p[1][:])
                szh2 = scal.tile([T, 1], f32, tag="szh2")
                nc.vector.tensor_add(szh2[:], szh_p[0][:], szh_p[1][:])

                sq_z = scal.tile([T, 1], f32, tag="sq_z")
                nc.scalar.activation(sq_z[:], ss_z[:], AF.Sqrt,
                                     bias=eps_t[:, :], scale=1.0 / D)
                rs_z = scal.tile([T, 1], f32, tag="rs_z")
                nc.vector.reciprocal(rs_z[:], sq_z[:])

                # w = z * rs_z + h (two chunks, feeding the PE transposes)
                w = work.tile([T, D], bf16, tag="w", bufs=2)
                for ci, (off, nn) in enumerate(zA_chunks):
                    nc.vector.scalar_tensor_tensor(
                        out=w[:, off:off + nn], in0=z_ps[ci][:],
                        scalar=rs_z[:, :], in1=h[:, off:off + nn],
                        op0=AL.mult, op1=AL.add,
                    )
                if i < NL - 1:
                    pT_sb = transpose_to_sbuf(w, f"w{i}")

                # ss_w = rs_z^2*ss_z + 2*rs_z*szh + ssh  (tiny [T,1] ops)
                v1 = scal.tile([T, 1], f32, tag="v1")
                nc.vector.scalar_tensor_tensor(
                    out=v1[:], in0=ss_z[:], scalar=rs_z[:, :], in1=szh2[:],
                    op0=AL.mult, op1=AL.add,
                )
                ss_w = scal.tile([T, 1], f32, tag="ss_w")
                nc.vector.scalar_tensor_tensor(
                    out=ss_w[:], in0=v1[:], scalar=rs_z[:, :], in1=ssh[:],
                    op0=AL.mult, op1=AL.add,
                )
                sq_w = scal.tile([T, 1], f32, tag="sq_w")
                nc.scalar.activation(sq_w[:], ss_w[:], AF.Sqrt,
                                     bias=eps_t[:, :], scale=1.0 / D)
                rs_w = scal.tile([T, 1], f32, tag="rs_w", bufs=2,
                                 name=f"rs_w{i}")
                nc.vector.reciprocal(rs_w[:], sq_w[:])

                if i < NL - 1:
                    h_next = work.tile([T, D], bf16, tag="h", bufs=2)
                    nc.vector.scalar_tensor_tensor(
                        out=h_next[:], in0=w[:], scalar=rs_w[:, :],
                        in1=Sb[i + 1][:], op0=AL.mult, op1=AL.add,
                    )
                    h = h_next
                    ssh = scal.tile([T, 1], f32, tag="ssh", bufs=2)
                    scrh = work.tile([T, D], bf16, tag="scr", bufs=2)
                    nc.scalar.activation(scrh[:], h[:], AF.Square,
                                         accum_out=ssh[:])
                else:
                    out_f = work.tile([T, D], f32, tag="out_f", bufs=1)
                    nc.vector.tensor_scalar_mul(out_f[:], w[:], rs_w[:, :])
                    nc.sync.dma_start(out_d[:, :], out_f[:])

    nc.compile()
    return nc


def _host_prep(x, in_proj_base, lora_A, lora_B, A_theta, B_real, B_imag,
               C_real, C_imag, out_proj_w, step_emb):
    W_in = in_proj_base.astype(np.float64) + 2.0 * (
        lora_B.astype(np.float64) @ lora_A.astype(np.float64))
    winT = np.ascontiguousarray(W_in.T).astype(ml_dtypes.bfloat16)
    woutT = np.ascontiguousarray(out_proj_w.T).astype(ml_dtypes.bfloat16)

    th = A_theta.astype(np.float64)
    P = (C_real.astype(np.float64) * B_real.astype(np.float64)
         - C_imag.astype(np.float64) * B_imag.astype(np.float64))
    Q = (C_real.astype(np.float64) * B_imag.astype(np.float64)
         + C_imag.astype(np.float64) * B_real.astype(np.float64))
    g4_f64 = np.stack([
        (P * np.cos(m * th) - Q * np.sin(m * th)).sum(-1).reshape(-1)
        for m in range(NL)
    ])                                                       # [4, 1536]
    g4 = g4_f64.astype(ml_dtypes.bfloat16)
    s4 = np.ascontiguousarray(step_emb).astype(ml_dtypes.bfloat16)

    # sW_k = step_emb[k] @ W_in^T; db_j = sum_{k=1..j} G_{j-k} * sW_k;
    # zdb_j = db_j @ W_out^T  (constant per-loop rank-1 rows, injected into
    # MM2's PSUM). Loop 0 has no step contribution (h0 = x + s0 explicit).
    sW = step_emb.astype(np.float64) @ W_in.T                # [4, F]
    zdb = np.zeros((NL, D))
    for j in range(1, NL):
        db_j = np.zeros(F)
        for k in range(1, j + 1):
            db_j += g4_f64[j - k] * sW[k]
        zdb[j] = db_j @ out_proj_w.astype(np.float64).T
    return winT, woutT, g4, s4, zdb.astype(ml_dtypes.bfloat16)


def kernel(x, in_proj_base, lora_A, lora_B, A_theta, B_real, B_imag,
           C_real, C_imag, out_proj_w, mixer_norm_w, loop_norm_w, step_emb,
           _trace=False):
    x = np.asarray(x, dtype=np.float32)
    winT, woutT, g4, s4, zdb = _host_prep(
        np.asarray(x), np.asarray(in_proj_base), np.asarray(lora_A),
        np.asarray(lora_B), np.asarray(A_theta), np.asarray(B_real),
        np.asarray(B_imag), np.asarray(C_real), np.asarray(C_imag),
        np.asarray(out_proj_w), np.asarray(step_emb))
    # mixer_norm_w / loop_norm_w are ones per the problem spec; rmsnorm weight
    # multiplies are identity and omitted on device.

    if "nc" not in _CACHE:
        _CACHE["nc"] = build_nc()
    nc = _CACHE["nc"]

    x16 = x.astype(ml_dtypes.bfloat16)
    shared = {"winT": winT, "woutT": woutT, "g4": g4, "s4": s4, "zdb": zdb}
    in_maps = [
        {**shared, "x_in": np.ascontiguousarray(x16[0, T * c:T * (c + 1), :])}
        for c in range(NCORES)
    ]
    res = run_bass_kernel_spmd(nc, in_maps, list(range(NCORES)), trace=_trace)
    out = np.concatenate(
        [np.asarray(res.results[c]["x_out"]) for c in range(NCORES)], axis=0)
    if _trace:
        _CACHE["last_result"] = res
    return out[None, :, :].astype(np.float32)


# revision 11
# speedup vs baseline: 1.5279x; 1.0643x over previous
"""Trainium2 Bass kernel for RecursiveMamba130M.

Math: the complex SSM state never needs materializing. With
  R = cos(theta) + j sin(theta),  Bc = Br + j Bi,  Cc = Cr + j Ci,
the per-loop output collapses to
  y_i[t, f] = sum_{k<=i} G_{i-k}[f] * u_k[t, f],   u_k = h_k @ W_in^T
where G_m[f] = sum_s Re(Cc * R^m * Bc).

Algebraic folds that keep the PE dense:
  * h_{i+1} = rs_w*w + step  =>  u_{i+1} = rs_w*(w @ W_in^T) + step@W_in^T.
    The per-token scale rs_w commutes through the matmul, so the PE
    transposes w (available right after rs_z) instead of h, and the
    rs_w scale rides the ACT PSUM->SBUF evacuation for free.
  * The step@W_in^T terms are constant rows; their contribution to z is
    zdb_i = (sum_k G_{i-k}*sW_k) @ W_out^T, injected into MM2's PSUM
    accumulation as a rank-1 ones-matmul. y on device is pure
    G0*u' + acc.
  * sum w^2 = rs_z^2*sum z^2 + 2*rs_z*sum(z*h) + sum h^2 (norm tail
    shortened; sum(z*h) on DVE in parallel with sum z^2 on ACT).

Sharding: data-parallel over the 1024 positions (128 tokens/core, no
collectives); weights replicated, all matmul data bf16 (fp32 PSUM,
fp32 norm statistics). MM1 is chunk-major so each 512-wide PSUM chunk
retires early and its evacuate/combine/transpose pipeline overlaps the
remaining matmuls; MM2 runs as two accumulation groups (512/256) so the
norm partials of the wide chunk hide under the narrow chunk's matmuls.
"""

import numpy as np
import ml_dtypes

import concourse.bass as bass
import concourse.tile as tile
from concourse.bacc import Bacc
from concourse import masks, mybir
from concourse.bass_utils import run_bass_kernel_spmd

T = 128          # tokens per core
D = 768          # d_model
F = 1536         # 2 * d_model
NL = 4           # reasoning loops
NCORES = 8
EPS = 1e-6

f32 = mybir.dt.float32
bf16 = mybir.dt.bfloat16
AL = mybir.AluOpType
AF = mybir.ActivationFunctionType

Z_CHUNKS = ((0, 512), (512, 256))

_CACHE = {}


def _act_rsqrt(nc, out, in_, bias_ap, scale):
    """out = Rsqrt(in_*scale + bias) on ScalarE.

    The bass wrapper refuses Rsqrt over precision concerns far below this
    kernel's 2e-2 tolerance; emit the instruction directly.
    """
    eng = nc.scalar
    ins = [
        eng.lower_ap(in_),
        eng.lower_ap(bias_ap),
        mybir.ImmediateValue(dtype=mybir.dt.float32, value=float(scale)),
        mybir.ImmediateValue(dtype=mybir.dt.float32, value=0.0),
    ]
    return eng.add_instruction(
        mybir.InstActivation(
            name=nc.get_next_instruction_name(),
            func=AF.Rsqrt,
            ins=ins,
            outs=[eng.lower_ap(out)],
        )
    )


def build_nc():
    nc = Bacc()
    x_d = nc.dram_tensor("x_in", [T, D], bf16, kind="ExternalInput")
    winT_d = nc.dram_tensor("winT", [D, F], bf16, kind="ExternalInput")
    woutT_d = nc.dram_tensor("woutT", [F, D], bf16, kind="ExternalInput")
    g4_d = nc.dram_tensor("g4", [NL, F], bf16, kind="ExternalInput")
    s4_d = nc.dram_tensor("s4", [NL, D], bf16, kind="ExternalInput")
    zdb_d = nc.dram_tensor("zdb", [NL, D], bf16, kind="ExternalInput")
    out_d = nc.dram_tensor("x_out", [T, D], f32, kind="ExternalOutput")

    with tile.TileContext(nc) as tc:
        with (
            tc.tile_pool(name="wpool", bufs=1) as wpool,
            tc.tile_pool(name="apool", bufs=1) as apool,
            tc.tile_pool(name="work", bufs=2) as work,
            tc.tile_pool(name="scal", bufs=1) as scal,
            tc.tile_pool(name="ps_u", bufs=1, space="PSUM") as ps_u,
            tc.tile_pool(name="ps_z", bufs=1, space="PSUM") as ps_z,
            tc.tile_pool(name="ps_t", bufs=1, space="PSUM") as ps_t,
            tc.tile_pool(name="ps_y", bufs=1, space="PSUM") as ps_y,
        ):
            # ---------- constants ----------
            ident = wpool.tile([128, 128], bf16, tag="ident")
            masks.make_identity(nc, ident[:])
            ones1 = wpool.tile([1, 128], bf16, tag="ones1")
            nc.vector.memset(ones1[:].bitcast(mybir.dt.uint32), 0x3F803F80)
            eps_t = wpool.tile([T, 1], f32, tag="eps_t")
            nc.vector.memset(eps_t[:], EPS)

            # ---------- DMAs (order = priority) ----------
            x_sb = wpool.tile([T, D], bf16, tag="x_sb")
            nc.sync.dma_start(x_sb[:], x_d[:, :])

            # all small rows land in partition 0 of two packed tiles
            rows_g = wpool.tile([1, NL, F], bf16, tag="rows_g")
            nc.sync.dma_start(rows_g[:], g4_d.rearrange("r n -> () r n"))
            rows_sz = wpool.tile([1, 2 * NL, D], bf16, tag="rows_sz")
            nc.sync.dma_start(rows_sz[:, 0:NL, :],
                              s4_d.rearrange("r n -> () r n"))
            nc.sync.dma_start(rows_sz[:, NL:2 * NL, :],
                              zdb_d.rearrange("r n -> () r n"))

            winT_sb = wpool.tile([128, 6, F], bf16, tag="winT_sb")
            for k in range(6):
                nc.sync.dma_start(winT_sb[:, k, :],
                                  winT_d[128 * k:128 * (k + 1), :])
            woutT_sb = wpool.tile([128, 12, D], bf16, tag="woutT_sb")
            for g in range(4):
                nc.sync.dma_start(
                    woutT_sb[:, 3 * g:3 * (g + 1), :],
                    woutT_d[384 * g:384 * (g + 1), :].rearrange(
                        "(k p) n -> p k n", p=128))

            # ---------- broadcast tiles via K=1 ones-matmul ----------
            # evacuation split across DVE/ACT (both idle in the prologue)
            def bcast_build(dst, row_ap, chunks, eng_copy, label):
                for ci, (off, nn) in enumerate(chunks):
                    if nn == 512:
                        pt = ps_u.tile([T, 512], f32, tag=f"u{ci}",
                                       name=f"bc_{label}_{ci}")
                    else:
                        pt = ps_z.tile([T, nn], f32, tag=f"z{ci}",
                                       name=f"bc_{label}_{ci}")
                    nc.tensor.matmul(pt[:], ones1[:, :],
                                     row_ap[:, off:off + nn],
                                     start=True, stop=True)
                    eng_copy(dst[:, off:off + nn], pt[:])

            F_CHUNKS = ((0, 512), (512, 512), (1024, 512))
            Sb, Gb = [], []
            for i in range(NL):
                sb = wpool.tile([T, D], bf16, tag=f"Sb{i}", name=f"Sb{i}")
                eng = nc.vector.tensor_copy if i == 0 else nc.scalar.copy
                bcast_build(sb, rows_sz[:, i, :], Z_CHUNKS, eng, f"sb{i}")
                Sb.append(sb)
            for m in range(NL):
                gb = wpool.tile([T, F], bf16, tag=f"Gb{m}", name=f"Gb{m}")
                eng = (nc.vector.tensor_copy if m in (0, 1)
                       else nc.scalar.copy)
                bcast_build(gb, rows_g[:, m, :], F_CHUNKS, eng, f"gb{m}")
                Gb.append(gb)

            # ---------- h0 = x + Sb0; transpose h0 on PE ----------
            h = work.tile([T, D], bf16, tag="h", bufs=2)
            nc.vector.tensor_add(h[:], x_sb[:], Sb[0][:])
            ssh = scal.tile([T, 1], f32, tag="ssh", bufs=2)
            scr0 = work.tile([T, D], bf16, tag="scr", bufs=2)
            nc.scalar.activation(scr0[:], h[:], AF.Square, accum_out=ssh[:])

            def transpose_to_sbuf(src, label, splits=((0, 6),)):
                """PE-transpose src [T, 768] (k-tile major) -> [T, 768] bf16
                via one-bank bf16 PSUM, DVE evacuation per half."""
                t_ps = ps_t.tile([T, 6, 128], bf16, tag="t",
                                 name=f"tps_{label}")
                for lo, hi in splits:
                    for k in range(lo, hi):
                        nc.tensor.transpose(
                            t_ps[:, k, :],
                            src[:, 128 * k:128 * (k + 1)],
                            ident[:],
                        )
                t_sb = work.tile([T, D], bf16, tag="pT_sb", bufs=2,
                                 name=f"tsb_{label}")
                for half in range(2):
                    nc.vector.tensor_copy(
                        t_sb[:, 384 * half:384 * (half + 1)],
                        t_ps[:, 3 * half:3 * (half + 1), :])
                return t_sb

            pT_sb = transpose_to_sbuf(h, "h0")

            accs = {}
            for j in (1, 2, 3):
                accs[j] = apool.tile([T, F], bf16, tag=f"acc{j}",
                                     name=f"acc{j}")

            rs_w = None
            # ---------- main loop ----------
            for i in range(NL):
                # MM1: p = (h|w) @ W_in^T. Loop 0 is DMA-paced: k-major so
                # each winT chunk is consumed as it lands. Steady loops are
                # chunk-major so chunk n retires after its 6 matmuls.
                u_ps = [ps_u.tile([T, 512], f32, tag=f"u{n}",
                                  name=f"u{i}_{n}") for n in range(3)]
                mm1_order = (
                    [(n, k) for k in range(6) for n in range(3)] if i == 0
                    else [(n, k) for n in range(3) for k in range(6)])
                for n, k in mm1_order:
                    nc.tensor.matmul(
                        u_ps[n][:],
                        pT_sb[:, 128 * k:128 * (k + 1)],
                        winT_sb[:, k, 512 * n:512 * (n + 1)],
                        start=(k == 0), stop=(k == 5),
                    )

                # u' = rs_w * p rides the ACT evacuation (plain copy at i=0);
                # y = G0*u' (+ acc_i); yT on PE right behind each y chunk
                u_sb = work.tile([T, F], bf16, tag="u_sb", bufs=2)
                y = work.tile([T, F], bf16, tag="y", bufs=2)
                yT_ps = ps_y.tile([T, 12, 128], bf16, tag="yt")
                yT_sb = work.tile([128, 12, 128], bf16, tag="yT_sb", bufs=2)
                for n in range(3):
                    sl = slice(512 * n, 512 * (n + 1))
                    if i == 0:
                        nc.scalar.copy(u_sb[:, sl], u_ps[n][:])
                    else:
                        nc.scalar.activation(u_sb[:, sl], u_ps[n][:],
                                             AF.Copy, scale=rs_w[:, :])
                    nc.vector.tensor_mul(y[:, sl], u_sb[:, sl], Gb[0][:, sl])
                    if i > 0:
                        nc.vector.tensor_add(y[:, sl], y[:, sl],
                                             accs[i][:, sl])
                    for c in range(4 * n, 4 * (n + 1)):
                        nc.tensor.transpose(
                            yT_ps[:, c, :],
                            y[:, 128 * c:128 * (c + 1)],
                            ident[:],
                        )
                    nc.vector.tensor_copy(yT_sb[:, 4 * n:4 * (n + 1), :],
                                          yT_ps[:, 4 * n:4 * (n + 1), :])

                # MM2: z = y @ out_proj^T (+ ones x zdb_i), A(512) then B(256)
                z_ps = []
                for ci, (off, nn) in enumerate(Z_CHUNKS):
                    zt = ps_z.tile([T, nn], f32, tag=f"z{ci}",
                                   name=f"z{i}_{ci}")
                    if i > 0:
                        nc.tensor.matmul(
                            zt[:], ones1[:, :],
                            rows_sz[:, NL + i, off:off + nn],
                            start=True, stop=False)
                    for c in range(12):
                        nc.tensor.matmul(
                            zt[:],
                            yT_sb[:, c, :],
                            woutT_sb[:, c, off:off + nn],
                            start=(c == 0 and i == 0), stop=(c == 11),
                        )
                    z_ps.append(zt)

                    # norm partials right behind each chunk's stop: the A
                    # partials run under B's matmuls
                    ssp = scal.tile([T, 1], f32, tag=f"ssz{ci}")
                    scr = work.tile([T, 512], bf16, tag="scr5", bufs=2)
                    nc.scalar.activation(scr[:, 0:nn], zt[:], AF.Square,
                                         accum_out=ssp[:])
                    shp = scal.tile([T, 1], f32, tag=f"szh{ci}")
                    zscr = work.tile([T, 512], bf16, tag="zscr", bufs=2)
                    nc.vector.scalar_tensor_tensor(
                        out=zscr[:, 0:nn], in0=zt[:], scalar=2.0,
                        in1=h[:, off:off + nn], op0=AL.mult, op1=AL.mult,
                        accum_out=shp[:],
                    )
                    if ci == 0:
                        ssz_A, szh_A = ssp, shp
                    else:
                        ss_z = scal.tile([T, 1], f32, tag="ss_z")
                        nc.vector.tensor_add(ss_z[:], ssz_A[:], ssp[:])
                        szh2 = scal.tile([T, 1], f32, tag="szh2")
                        nc.vector.tensor_add(szh2[:], szh_A[:], shp[:])

                rs_z = scal.tile([T, 1], f32, tag="rs_z")
                _act_rsqrt(nc, rs_z[:], ss_z[:], eps_t[:, :], 1.0 / D)

                # w = z * rs_z + h, then transpose w for the next MM1
                w = work.tile([T, D], bf16, tag="w", bufs=2)
                for ci, (off, nn) in enumerate(Z_CHUNKS):
                    nc.vector.scalar_tensor_tensor(
                        out=w[:, off:off + nn], in0=z_ps[ci][:],
                        scalar=rs_z[:, :], in1=h[:, off:off + nn],
                        op0=AL.mult, op1=AL.add,
                    )
                if i < NL - 1:
                    pT_sb = transpose_to_sbuf(w, f"w{i}",
                                              splits=((0, 4), (4, 6)))

                # ss_w = rs_z^2*ss_z + 2*rs_z*szh + ssh  (tiny [T,1] ops)
                v1 = scal.tile([T, 1], f32, tag="v1")
                nc.vector.scalar_tensor_tensor(
                    out=v1[:], in0=ss_z[:], scalar=rs_z[:, :], in1=szh2[:],
                    op0=AL.mult, op1=AL.add,
                )
                ss_w = scal.tile([T, 1], f32, tag="ss_w")
                nc.vector.scalar_tensor_tensor(
                    out=ss_w[:], in0=v1[:], scalar=rs_z[:, :], in1=ssh[:],
                    op0=AL.mult, op1=AL.add,
                )
                rs_w = scal.tile([T, 1], f32, tag="rs_w", bufs=2,
                                 name=f"rs_w{i}")
                _act_rsqrt(nc, rs_w[:], ss_w[:], eps_t[:, :], 1.0 / D)

                if i < NL - 1:
                    h_next = work.tile([T, D], bf16, tag="h", bufs=2)
                    nc.vector.scalar_tensor_tensor(
                        out=h_next[:], in0=w[:], scalar=rs_w[:, :],
                        in1=Sb[i + 1][:], op0=AL.mult, op1=AL.add,
                    )
                    h = h_next
                    ssh = scal.tile([T, 1], f32, tag="ssh", bufs=2)
                    scrh = work.tile([T, D], bf16, tag="scr", bufs=2)
                    nc.scalar.activation(scrh[:], h[:], AF.Square,
                                         accum_out=ssh[:])
                else:
                    out_f = work.tile([T, D], f32, tag="out_f", bufs=1)
                    nc.vector.tensor_scalar_mul(out_f[:], w[:], rs_w[:, :])
                    nc.sync.dma_start(out_d[:, :], out_f[:])

                # acc updates last (deep slack; they fill next loop's MM1
                # window on DVE)
                for j in range(i + 1, NL):
                    m = j - i
                    if i == 0:
                        nc.vector.tensor_mul(accs[j][:], u_sb[:], Gb[m][:])
                    else:
                        tmp_a = work.tile([T, F], bf16, tag="tmp_a", bufs=2)
                        nc.vector.tensor_mul(tmp_a[:], u_sb[:], Gb[m][:])
                        nc.vector.tensor_add(accs[j][:], accs[j][:],
                                             tmp_a[:])

    nc.compile()
    return nc


def _host_prep(x, in_proj_base, lora_A, lora_B, A_theta, B_real, B_imag,
               C_real, C_imag, out_proj_w, step_emb):
    W_in = in_proj_base.astype(np.float64) + 2.0 * (
        lora_B.astype(np.float64) @ lora_A.astype(np.float64))
    winT = np.ascontiguousarray(W_in.T).astype(ml_dtypes.bfloat16)
    woutT = np.ascontiguousarray(out_proj_w.T).astype(ml_dtypes.bfloat16)

    th = A_theta.astype(np.float64)
    P = (C_real.astype(np.float64) * B_real.astype(np.float64)
         - C_imag.astype(np.float64) * B_imag.astype(np.float64))
    Q = (C_real.astype(np.float64) * B_imag.astype(np.float64)
         + C_imag.astype(np.float64) * B_real.astype(np.float64))
    g4_f64 = np.stack([
        (P * np.cos(m * th) - Q * np.sin(m * th)).sum(-1).reshape(-1)
        for m in range(NL)
    ])                                                       # [4, 1536]
    g4 = g4_f64.astype(ml_dtypes.bfloat16)
    s4 = np.ascontiguousarray(step_emb).astype(ml_dtypes.bfloat16)

    # sW_k = step_emb[k] @ W_in^T; db_j = sum_{k=1..j} G_{j-k} * sW_k;
    # zdb_j = db_j @ W_out^T  (constant rank-1 rows injected into MM2's
    # PSUM). Loop 0 has no step contribution (h0 = x + s0 explicit).
    sW = step_emb.astype(np.float64) @ W_in.T                # [4, F]
    zdb = np.zeros((NL, D))
    for j in range(1, NL):
        db_j = np.zeros(F)
        for k in range(1, j + 1):
            db_j += g4_f64[j - k] * sW[k]
        zdb[j] = db_j @ out_proj_w.astype(np.float64).T
    return winT, woutT, g4, s4, zdb.astype(ml_dtypes.bfloat16)


def kernel(x, in_proj_base, lora_A, lora_B, A_theta, B_real, B_imag,
           C_real, C_imag, out_proj_w, mixer_norm_w, loop_norm_w, step_emb,
           _trace=False):
    x = np.asarray(x, dtype=np.float32)
    winT, woutT, g4, s4, zdb = _host_prep(
        np.asarray(x), np.asarray(in_proj_base), np.asarray(lora_A),
        np.asarray(lora_B), np.asarray(A_theta), np.asarray(B_real),
        np.asarray(B_imag), np.asarray(C_real), np.asarray(C_imag),
        np.asarray(out_proj_w), np.asarray(step_emb))
    # mixer_norm_w / loop_norm_w are ones per the problem spec; rmsnorm weight
    # multiplies are identity and omitted on device.

    if "nc" not in _CACHE:
        _CACHE["nc"] = build_nc()
    nc = _CACHE["nc"]

    x16 = x.astype(ml_dtypes.bfloat16)
    shared = {"winT": winT, "woutT": woutT, "g4": g4, "s4": s4, "zdb": zdb}
    in_maps = [
        {**shared, "x_in": np.ascontiguousarray(x16[0, T * c:T * (c + 1), :])}
        for c in range(NCORES)
    ]
    res = run_bass_kernel_spmd(nc, in_maps, list(range(NCORES)), trace=_trace)
    out = np.concatenate(
        [np.asarray(res.results[c]["x_out"]) for c in range(NCORES)], axis=0)
    if _trace:
        _CACHE["last_result"] = res
    return out[None, :, :].astype(np.float32)


# revision 13
# speedup vs baseline: 1.6124x; 1.0553x over previous
"""Trainium2 Bass kernel for RecursiveMamba130M.

Math: the complex SSM state never needs materializing. With
  R = cos(theta) + j sin(theta),  Bc = Br + j Bi,  Cc = Cr + j Ci,
the per-loop output collapses to
  y_i[t, f] = sum_{k<=i} G_{i-k}[f] * u_k[t, f],   u_k = h_k @ W_in^T
where G_m[f] = sum_s Re(Cc * R^m * Bc).

Algebraic folds that keep the PE dense:
  * h_{i+1} = rs_w*w + step  =>  u_{i+1} = rs_w*(w @ W_in^T) + step@W_in^T.
    The per-token scale rs_w commutes through the matmul, so the PE
    transposes w (available right after rs_z) instead of h, and the
    rs_w scale rides the ACT PSUM->SBUF evacuation for free.
  * The step@W_in^T terms are constant rows; their contribution to z is
    zdb_i = (sum_k G_{i-k}*sW_k) @ W_out^T, injected into MM2's PSUM
    accumulation as a rank-1 ones-matmul. y on device is pure
    G0*u' + acc.
  * sum w^2 = rs_z^2*sum z^2 + 2*rs_z*sum(z*h) + sum h^2 (norm tail
    shortened; sum(z*h) on DVE in parallel with sum z^2 on ACT).

Sharding: data-parallel over the 1024 positions (128 tokens/core, no
collectives); weights replicated, all matmul data bf16 (fp32 PSUM,
fp32 norm statistics). MM1 is chunk-major so each 512-wide PSUM chunk
retires early and its evacuate/combine/transpose pipeline overlaps the
remaining matmuls; MM2 runs as two accumulation groups (512/256) so the
norm partials of the wide chunk hide under the narrow chunk's matmuls.
"""

import numpy as np
import ml_dtypes

import concourse.bass as bass
import concourse.tile as tile
from concourse.bacc import Bacc
from concourse import masks, mybir
from concourse.bass_utils import run_bass_kernel_spmd

T = 128          # tokens per core
D = 768          # d_model
F = 1536         # 2 * d_model
NL = 4           # reasoning loops
NCORES = 8
EPS = 1e-6

f32 = mybir.dt.float32
bf16 = mybir.dt.bfloat16
AL = mybir.AluOpType
AF = mybir.ActivationFunctionType

Z_CHUNKS = ((0, 512), (512, 256))

_CACHE = {}


def _act_rsqrt(nc, out, in_, bias_ap, scale):
    """out = Rsqrt(in_*scale + bias) on ScalarE.

    The bass wrapper refuses Rsqrt over precision concerns far below this
    kernel's 2e-2 tolerance; emit the instruction directly.
    """
    eng = nc.scalar
    ins = [
        eng.lower_ap(in_),
        eng.lower_ap(bias_ap),
        mybir.ImmediateValue(dtype=mybir.dt.float32, value=float(scale)),
        mybir.ImmediateValue(dtype=mybir.dt.float32, value=0.0),
    ]
    return eng.add_instruction(
        mybir.InstActivation(
            name=nc.get_next_instruction_name(),
            func=AF.Rsqrt,
            ins=ins,
            outs=[eng.lower_ap(out)],
        )
    )


def build_nc():
    nc = Bacc()
    x_d = nc.dram_tensor("x_in", [T, D], bf16, kind="ExternalInput")
    winT_d = nc.dram_tensor("winT", [D, F], bf16, kind="ExternalInput")
    woutT_d = nc.dram_tensor("woutT", [F, D], bf16, kind="ExternalInput")
    g4_d = nc.dram_tensor("g4", [NL, F], bf16, kind="ExternalInput")
    s4_d = nc.dram_tensor("s4", [NL, D], bf16, kind="ExternalInput")
    zdb_d = nc.dram_tensor("zdb", [NL, D], bf16, kind="ExternalInput")
    out_d = nc.dram_tensor("x_out", [T, D], f32, kind="ExternalOutput")

    with tile.TileContext(nc) as tc:
        with (
            tc.tile_pool(name="wpool", bufs=1) as wpool,
            tc.tile_pool(name="apool", bufs=1) as apool,
            tc.tile_pool(name="work", bufs=2) as work,
            tc.tile_pool(name="scal", bufs=1) as scal,
            tc.tile_pool(name="ps_u", bufs=1, space="PSUM") as ps_u,
            tc.tile_pool(name="ps_z", bufs=1, space="PSUM") as ps_z,
            tc.tile_pool(name="ps_t", bufs=1, space="PSUM") as ps_t,
            tc.tile_pool(name="ps_y", bufs=1, space="PSUM") as ps_y,
        ):
            # ---------- constants ----------
            ident = wpool.tile([128, 128], bf16, tag="ident")
            masks.make_identity(nc, ident[:])
            ones1 = wpool.tile([1, 128], bf16, tag="ones1")
            nc.vector.memset(ones1[:].bitcast(mybir.dt.uint32), 0x3F803F80)
            eps_t = wpool.tile([T, 1], f32, tag="eps_t")
            nc.vector.memset(eps_t[:], EPS)

            # ---------- DMAs (order = priority) ----------
            x_sb = wpool.tile([T, D], bf16, tag="x_sb")
            nc.sync.dma_start(x_sb[:], x_d[:, :])

            # all small rows land in partition 0 of two packed tiles
            rows_g = wpool.tile([1, NL, F], bf16, tag="rows_g")
            nc.sync.dma_start(rows_g[:], g4_d.rearrange("r n -> () r n"))
            rows_sz = wpool.tile([1, 2 * NL, D], bf16, tag="rows_sz")
            nc.sync.dma_start(rows_sz[:, 0:NL, :],
                              s4_d.rearrange("r n -> () r n"))
            nc.sync.dma_start(rows_sz[:, NL:2 * NL, :],
                              zdb_d.rearrange("r n -> () r n"))

            winT_sb = wpool.tile([128, 6, F], bf16, tag="winT_sb")
            for k in range(6):
                nc.sync.dma_start(winT_sb[:, k, :],
                                  winT_d[128 * k:128 * (k + 1), :])
            woutT_sb = wpool.tile([128, 12, D], bf16, tag="woutT_sb")
            for g in range(4):
                nc.sync.dma_start(
                    woutT_sb[:, 3 * g:3 * (g + 1), :],
                    woutT_d[384 * g:384 * (g + 1), :].rearrange(
                        "(k p) n -> p k n", p=128))

            # ---------- broadcast tiles via K=1 ones-matmul ----------
            # evacuation split across DVE/ACT (both idle in the prologue)
            def bcast_build(dst, row_ap, chunks, eng_copy, label):
                for ci, (off, nn) in enumerate(chunks):
                    if nn == 512:
                        pt = ps_u.tile([T, 512], f32, tag=f"u{ci}",
                                       name=f"bc_{label}_{ci}")
                    else:
                        pt = ps_z.tile([T, nn], f32, tag=f"z{ci}",
                                       name=f"bc_{label}_{ci}")
                    nc.tensor.matmul(pt[:], ones1[:, :],
                                     row_ap[:, off:off + nn],
                                     start=True, stop=True)
                    eng_copy(dst[:, off:off + nn], pt[:])

            F_CHUNKS = ((0, 512), (512, 512), (1024, 512))
            Sb, Gb = [], []
            for i in range(NL):
                sb = wpool.tile([T, D], bf16, tag=f"Sb{i}", name=f"Sb{i}")
                eng = nc.vector.tensor_copy if i == 0 else nc.scalar.copy
                bcast_build(sb, rows_sz[:, i, :], Z_CHUNKS, eng, f"sb{i}")
                Sb.append(sb)
            for m in range(NL):
                gb = wpool.tile([T, F], bf16, tag=f"Gb{m}", name=f"Gb{m}")
                eng = (nc.vector.tensor_copy if m in (0, 1)
                       else nc.scalar.copy)
                bcast_build(gb, rows_g[:, m, :], F_CHUNKS, eng, f"gb{m}")
                Gb.append(gb)

            # ---------- h0 = x + Sb0; transpose h0 on PE ----------
            h = work.tile([T, D], bf16, tag="h", bufs=2)
            nc.vector.tensor_add(h[:], x_sb[:], Sb[0][:])

            def transpose_to_sbuf(src, label, splits=((0, 6),)):
                """PE-transpose src [T, 768] (k-tile major) -> [T, 768] bf16
                via one-bank bf16 PSUM, DVE evacuation per half."""
                t_ps = ps_t.tile([T, 6, 128], bf16, tag="t",
                                 name=f"tps_{label}")
                for lo, hi in splits:
                    for k in range(lo, hi):
                        nc.tensor.transpose(
                            t_ps[:, k, :],
                            src[:, 128 * k:128 * (k + 1)],
                            ident[:],
                        )
                t_sb = work.tile([T, D], bf16, tag="pT_sb", bufs=2,
                                 name=f"tsb_{label}")
                for half in range(2):
                    nc.vector.tensor_copy(
                        t_sb[:, 384 * half:384 * (half + 1)],
                        t_ps[:, 3 * half:3 * (half + 1), :])
                return t_sb

            pT_sb = transpose_to_sbuf(h, "h0")

            accs = {}
            for j in (1, 2, 3):
                accs[j] = apool.tile([T, F], bf16, tag=f"acc{j}",
                                     name=f"acc{j}")

            rs_w = None
            # ---------- main loop ----------
            for i in range(NL):
                # MM1: p = (h|w) @ W_in^T. Loop 0 is DMA-paced: k-major so
                # each winT chunk is consumed as it lands. Steady loops are
                # chunk-major so chunk n retires after its 6 matmuls.
                u_ps = [ps_u.tile([T, 512], f32, tag=f"u{n}",
                                  name=f"u{i}_{n}") for n in range(3)]
                mm1_order = (
                    [(n, k) for k in range(6) for n in range(3)] if i == 0
                    else [(n, k) for n in range(3) for k in range(6)])
                for n, k in mm1_order:
                    nc.tensor.matmul(
                        u_ps[n][:],
                        pT_sb[:, 128 * k:128 * (k + 1)],
                        winT_sb[:, k, 512 * n:512 * (n + 1)],
                        start=(k == 0), stop=(k == 5),
                    )

                # u' = rs_w * p rides the ACT evacuation (plain copy at i=0);
                # y = G0*u' (+ acc_i); yT on PE right behind each y chunk
                u_sb = work.tile([T, F], bf16, tag="u_sb", bufs=2)
                y = work.tile([T, F], bf16, tag="y", bufs=2)
                yT_ps = ps_y.tile([T, 12, 128], bf16, tag="yt")
                yT_sb = work.tile([128, 12, 128], bf16, tag="yT_sb", bufs=2)
                for n in range(3):
                    sl = slice(512 * n, 512 * (n + 1))
                    if i == 0:
                        nc.scalar.copy(u_sb[:, sl], u_ps[n][:])
                    else:
                        nc.scalar.activation(u_sb[:, sl], u_ps[n][:],
                                             AF.Copy, scale=rs_w[:, :])
                    nc.vector.tensor_mul(y[:, sl], u_sb[:, sl], Gb[0][:, sl])
                    if i > 0:
                        nc.vector.tensor_add(y[:, sl], y[:, sl],
                                             accs[i][:, sl])
                    for c in range(4 * n, 4 * (n + 1)):
                        nc.tensor.transpose(
                            yT_ps[:, c, :],
                            y[:, 128 * c:128 * (c + 1)],
                            ident[:],
                        )
                    nc.vector.tensor_copy(yT_sb[:, 4 * n:4 * (n + 1), :],
                                          yT_ps[:, 4 * n:4 * (n + 1), :])

                # MM2: z = y @ out_proj^T (+ ones x zdb_i), A(512) then B(256)
                z_ps = []
                for ci, (off, nn) in enumerate(Z_CHUNKS):
                    zt = ps_z.tile([T, nn], f32, tag=f"z{ci}",
                                   name=f"z{i}_{ci}")
                    if i > 0:
                        nc.tensor.matmul(
                            zt[:], ones1[:, :],
                            rows_sz[:, NL + i, off:off + nn],
                            start=True, stop=False)
                    for c in range(12):
                        nc.tensor.matmul(
                            zt[:],
                            yT_sb[:, c, :],
                            woutT_sb[:, c, off:off + nn],
                            start=(c == 0 and i == 0), stop=(c == 11),
                        )
                    z_ps.append(zt)

                    # norm partials right behind each chunk's stop: the A
                    # partial runs under B's matmuls; the partial add stays
                    # on ACT (Identity+bias) to keep the DVE free for w
                    ssp = scal.tile([T, 1], f32, tag=f"ssz{ci}")
                    scr = work.tile([T, 512], bf16, tag="scr5", bufs=2)
                    nc.scalar.activation(scr[:, 0:nn], zt[:], AF.Square,
                                         accum_out=ssp[:])
                    if ci == 0:
                        ssz_A = ssp
                        scrA = scr
                    else:
                        ss_z = scal.tile([T, 1], f32, tag="ss_z")
                        nc.scalar.activation(ss_z[:], ssz_A[:], AF.Identity,
                                             bias=ssp[:, :])

                rs_z = scal.tile([T, 1], f32, tag="rs_z")
                _act_rsqrt(nc, rs_z[:], ss_z[:], eps_t[:, :], 1.0 / D)

                # HAM keep-alive: one tiny real matmul mid-tail (reads the
                # Square scratch, so it fires between MM2 and the w
                # transposes) keeps the PE activity monitor from
                # re-throttling the clock during the norm chain
                dscr = ps_u.tile([T, 64], f32, tag="u0", name=f"dummy{i}")
                nc.tensor.matmul(dscr[:], scrA[:, 0:128], ident[:, 0:64],
                                 start=True, stop=True)

                # w = z * rs_z + h, then transpose w for the next MM1
                w = work.tile([T, D], bf16, tag="w", bufs=2)
                for ci, (off, nn) in enumerate(Z_CHUNKS):
                    nc.vector.scalar_tensor_tensor(
                        out=w[:, off:off + nn], in0=z_ps[ci][:],
                        scalar=rs_z[:, :], in1=h[:, off:off + nn],
                        op0=AL.mult, op1=AL.add,
                    )
                if i < NL - 1:
                    pT_sb = transpose_to_sbuf(w, f"w{i}",
                                              splits=((0, 4), (4, 6)))

                # ss_w via direct Square(w) on ACT (off the PE path: rs_w
                # is only needed by the next loop's u' evacuation)
                ss_w = scal.tile([T, 1], f32, tag="ss_w")
                scrw = work.tile([T, D], bf16, tag="scr", bufs=2)
                nc.scalar.activation(scrw[:], w[:], AF.Square,
                                     accum_out=ss_w[:])
                rs_w = scal.tile([T, 1], f32, tag="rs_w", bufs=2,
                                 name=f"rs_w{i}")
                _act_rsqrt(nc, rs_w[:], ss_w[:], eps_t[:, :], 1.0 / D)

                if i < NL - 1:
                    h_next = work.tile([T, D], bf16, tag="h", bufs=2)
                    nc.vector.scalar_tensor_tensor(
                        out=h_next[:], in0=w[:], scalar=rs_w[:, :],
                        in1=Sb[i + 1][:], op0=AL.mult, op1=AL.add,
                    )
                    h = h_next
                else:
                    out_f = work.tile([T, D], f32, tag="out_f", bufs=1)
                    nc.vector.tensor_scalar_mul(out_f[:], w[:], rs_w[:, :])
                    nc.sync.dma_start(out_d[:, :], out_f[:])

                # acc updates last (deep slack; they fill next loop's MM1
                # window on DVE)
                for j in range(i + 1, NL):
                    m = j - i
                    if i == 0:
                        nc.vector.tensor_mul(accs[j][:], u_sb[:], Gb[m][:])
                    else:
                        tmp_a = work.tile([T, F], bf16, tag="tmp_a", bufs=2)
                        nc.vector.tensor_mul(tmp_a[:], u_sb[:], Gb[m][:])
                        nc.vector.tensor_add(accs[j][:], accs[j][:],
                                             tmp_a[:])

    nc.compile()
    return nc


def _host_prep(x, in_proj_base, lora_A, lora_B, A_theta, B_real, B_imag,
               C_real, C_imag, out_proj_w, step_emb):
    W_in = in_proj_base.astype(np.float64) + 2.0 * (
        lora_B.astype(np.float64) @ lora_A.astype(np.float64))
    winT = np.ascontiguousarray(W_in.T).astype(ml_dtypes.bfloat16)
    woutT = np.ascontiguousarray(out_proj_w.T).astype(ml_dtypes.bfloat16)

    th = A_theta.astype(np.float64)
    P = (C_real.astype(np.float64) * B_real.astype(np.float64)
         - C_imag.astype(np.float64) * B_imag.astype(np.float64))
    Q = (C_real.astype(np.float64) * B_imag.astype(np.float64)
         + C_imag.astype(np.float64) * B_real.astype(np.float64))
    g4_f64 = np.stack([
        (P * np.cos(m * th) - Q * np.sin(m * th)).sum(-1).reshape(-1)
        for m in range(NL)
    ])                                                       # [4, 1536]
    g4 = g4_f64.astype(ml_dtypes.bfloat16)
    s4 = np.ascontiguousarray(step_emb).astype(ml_dtypes.bfloat16)

    # sW_k = step_emb[k] @ W_in^T; db_j = sum_{k=1..j} G_{j-k} * sW_k;
    # zdb_j = db_j @ W_out^T  (constant rank-1 rows injected into MM2's
    # PSUM). Loop 0 has no step contribution (h0 = x + s0 explicit).
    sW = step_emb.astype(np.float64) @ W_in.T                # [4, F]
    zdb = np.zeros((NL, D))
    for j in range(1, NL):
        db_j = np.zeros(F)
        for k in range(1, j + 1):
            db_j += g4_f64[j - k] * sW[k]
        zdb[j] = db_j @ out_proj_w.astype(np.float64).T
    return winT, woutT, g4, s4, zdb.astype(ml_dtypes.bfloat16)


def kernel(x, in_proj_base, lora_A, lora_B, A_theta, B_real, B_imag,
           C_real, C_imag, out_proj_w, mixer_norm_w, loop_norm_w, step_emb,
           _trace=False):
    x = np.asarray(x, dtype=np.float32)
    winT, woutT, g4, s4, zdb = _host_prep(
        np.asarray(x), np.asarray(in_proj_base), np.asarray(lora_A),
        np.asarray(lora_B), np.asarray(A_theta), np.asarray(B_real),
        np.asarray(B_imag), np.asarray(C_real), np.asarray(C_imag),
        np.asarray(out_proj_w), np.asarray(step_emb))
    # mixer_norm_w / loop_norm_w are ones per the problem spec; rmsnorm weight
    # multiplies are identity and omitted on device.

    if "nc" not in _CACHE:
        _CACHE["nc"] = build_nc()
    nc = _CACHE["nc"]

    x16 = x.astype(ml_dtypes.bfloat16)
    shared = {"winT": winT, "woutT": woutT, "g4": g4, "s4": s4, "zdb": zdb}
    in_maps = [
        {**shared, "x_in": np.ascontiguousarray(x16[0, T * c:T * (c + 1), :])}
        for c in range(NCORES)
    ]
    res = run_bass_kernel_spmd(nc, in_maps, list(range(NCORES)), trace=_trace)
    out = np.concatenate(
        [np.asarray(res.results[c]["x_out"]) for c in range(NCORES)], axis=0)
    if _trace:
        _CACHE["last_result"] = res
    return out[None, :, :].astype(np.float32)
